# revision 19
# baseline (speedup 1.0000x reference)
"""Trainium2 Bass kernel for GPT-Neo style causal attention.

reference:
    scores = q @ k.T              (no 1/sqrt(d) scaling), fp32
    scores = where(causal, scores, -inf)
    attn   = softmax(scores, -1)
    attn   = attn * ctx_mask[b, None, None, :]
    out    = attn @ v

Shapes: B=2, H=16, S=2048, D=128 fp32. Sharded over 8 cores by (b*h) —
4 heads per core; each core's heads belong to one batch, so one
ctx_mask row per core.

Per-core algorithm (T-layout softmax, no transposes of the attn matrix):
  - load Q,K natural fp32, DVE-cast to fp16, then ONE xbar DMA-transpose
    per tensor (3D-out batched 128x128 transpose) -> interleaved
    [Q^T | K^T] tile [d, s] in fp16 (1 cyc/col matmuls + FWL weight
    loads; frees the PE of 32 transpose matmuls/head vs the old
    PE-transpose + fp32r pipeline)
  - per key-block t: scoresT[keys,q] = KT_blk.T @ QT  (only q >= t*128,
    512-col segments aligned to PSUM banks)
  - one exp() per strip on ScalarE with per-partition bias ln(ctx_mask):
    expT = exp(s + ln(cm_key)) = exp(s)*cm_key  -> bf16 (the ctx-mask
    multiply costs nothing).  Causal diag via upper-triangular 0/1 mul.
  - AV: out_psum[q, 0:129] = sum_kb expT_blk.T @ [V | 1/cm] (bf16,
    fp32 PSUM accum).  Column 128 accumulates exp*cm*(1/cm) = exp,
    i.e. the pre-ctx-mask softmax denominator -> reciprocal + scale.
  - cm clamped at 1e-30 so cm=0 stays exact (exp(s+ln(1e-30))*1e30 =
    exp(s) in the denominator, 0 in the numerator).

No max-subtraction is needed: |scores| <~ 70 so exp() stays inside fp32/
bf16 range (both share the 8-bit exponent), and softmax is shift-invariant.
A ~6us dummy bf16 matmul burst at the start (hidden under the first input
DMA) warms the PE HAM clock gate to 2.4 GHz.
"""

from contextlib import ExitStack

import numpy as np

import concourse.bass as bass
import concourse.mybir as mybir
import concourse.tile as tile
from concourse.bass_utils import run_bass_kernel_spmd
from concourse.masks import make_identity, make_lower_triangular, make_upper_triangular

F32 = mybir.dt.float32
F32R = mybir.dt.float32r
F16 = mybir.dt.float16
BF16 = mybir.dt.bfloat16

B, H, S, D = 2, 16, 2048, 128
NCORES = 8
NBH = (B * H) // NCORES  # heads per core


def _legalize_waits(nc):
    """This container's walrus accepts at most 1 sync wait per instruction
    (2 for EventSemaphore). Hoist extra waits onto same-engine NoOps
    inserted immediately before the offending instruction (semantically
    identical: all waits still complete before it executes)."""
    n = 0
    ctr = [0]
    for f in nc.m.functions:
        for bb in f.blocks:
            out = []
            dirty = False
            for inst in bb.instructions:
                si = inst.sync_info
                cap = 2 if isinstance(inst, mybir.InstEventSemaphore) else 1
                if si is not None and len(si.on_wait) > cap:
                    waits = list(si.on_wait)
                    extra, keep = waits[:-cap], waits[-cap:]
                    for w in extra:
                        ctr[0] += 1
                        nop = mybir.InstNoOp(
                            name=f"waitsplit-{ctr[0]}",
                            ins=[],
                            outs=[],
                            engine=inst.engine,
                            sync_info=mybir.SyncInfo(on_wait=[w], on_update=[]),
                        )
                        nc.register_instruction(nop, overwrite=True)
                        out.append(nop)
                    inst.sync_info = mybir.SyncInfo(
                        on_wait=keep, on_update=list(si.on_update)
                    )
                    dirty = True
                    n += 1
                out.append(inst)
            if dirty:
                bb.instructions = out
    return n


def build_nc(nbh=NBH, s=S, d=D, num_devices=NCORES):
    SB = s // 128  # 128-row blocks along the sequence
    nc = bass.Bass("TRN2", target_bir_lowering=False, debug=False,
                   num_devices=num_devices)
    q = nc.dram_tensor("q", [nbh, s, d], F32, kind="ExternalInput")
    k = nc.dram_tensor("k", [nbh, s, d], F32, kind="ExternalInput")
    v = nc.dram_tensor("v", [nbh, s, d], F32, kind="ExternalInput")
    cm = nc.dram_tensor("cm", [s], F32, kind="ExternalInput")
    o = nc.dram_tensor("out", [nbh, s, d], F32, kind="ExternalOutput")

    EXPFN = mybir.ActivationFunctionType.Exp
    LNFN = mybir.ActivationFunctionType.Ln

    with tile.TileContext(nc) as tc, ExitStack() as ctx:
        consts = ctx.enter_context(tc.tile_pool(name="consts", bufs=1))
        stage = ctx.enter_context(tc.tile_pool(name="stage", bufs=2))
        hpool = ctx.enter_context(tc.tile_pool(name="hpool", bufs=2))
        tpool = ctx.enter_context(tc.tile_pool(name="tpool", bufs=2))
        vpool = ctx.enter_context(tc.tile_pool(name="vpool", bufs=2))
        epool = ctx.enter_context(tc.tile_pool(name="epool", bufs=1))
        opool = ctx.enter_context(tc.tile_pool(name="opool", bufs=2))
        small = ctx.enter_context(tc.tile_pool(name="small", bufs=4))
        psum = ctx.enter_context(tc.tile_pool(name="psum", bufs=2, space="PSUM"))
        psav = ctx.enter_context(tc.tile_pool(name="psav", bufs=2, space="PSUM"))

        ident = consts.tile([128, 128], F32)
        make_identity(nc, ident)
        identb = consts.tile([128, 128], BF16)
        nc.vector.tensor_copy(identb, ident)
        # additive causal mask for the diagonal block, accumulated into the
        # scores PSUM by the PE itself: matmul(trinegT, I) adds
        # trinegT.T[k, q] = -3e38 for q < k. Keeps the DVE off the
        # scores->exp critical path (a pre-exp DVE tensor_add there stalls
        # every strip whenever the DVE queue is busy with casts).
        trinegT = consts.tile([128, 128], F32)
        make_upper_triangular(nc, trinegT, val=-3e38, diag=False)
        trinegTb = consts.tile([128, 128], BF16)
        nc.vector.tensor_copy(trinegTb, trinegT)

        # ctx-mask pipeline: cmc = max(cm, 1e-30); lncm = ln(cmc) (exp bias);
        # invc = 1/cmc in bf16 (denominator column of V')
        cmt = consts.tile([128, SB], F32)
        nc.sync.dma_start(out=cmt, in_=cm.ap().rearrange("(sb p) -> p sb", p=128))
        cmc = consts.tile([128, SB], F32)
        nc.vector.tensor_scalar_max(cmc, cmt, 1e-30)
        # -16 shift keeps exp() in fp32/bf16 range for the largest observed
        # scores (~95); it cancels exactly in the softmax ratio since the
        # denominator column scales identically.
        lncm = consts.tile([128, SB], F32)
        nc.scalar.activation(lncm, cmc, LNFN)
        nc.vector.tensor_scalar_add(lncm, lncm, -16.0)
        invc = consts.tile([128, SB], F32)
        nc.vector.reciprocal(invc, cmc)
        invcb = consts.tile([128, SB], BF16)
        nc.vector.tensor_copy(invcb, invc)

        # Dummy bf16 matmuls (values irrelevant) to warm the PE clock gate
        # while the first input DMAs are in flight; memset-only dep so the
        # burst starts at t~0.
        wpw = consts.tile([128, 128], BF16)
        nc.vector.memset(wpw, 1.0)
        wps = psav.tile([128, 256], F32, tag="av")
        for _ in range(210):
            nc.tensor.matmul(wps[:, 0:128], wpw, wpw, start=True, stop=True)

        qap, kap, vap, oap = q.ap(), k.ap(), v.ap(), o.ap()

        def load_in(ap, bh, tag):
            tile_ = stage.tile([128, SB, d], F32, tag=tag)
            nc.sync.dma_start(out=tile_,
                              in_=ap[bh].rearrange("(sb p) d -> p sb d", p=128))
            return tile_

        def cast_chunk(dst, src, half):
            h0 = (SB // 2) * half
            nc.vector.tensor_copy(dst[:, h0:h0 + SB // 2, :],
                                  src[:, h0:h0 + SB // 2, :])

        def transpose_qkt(qkt, slot, src):
            # batched per-sb 128x128 xbar transpose (3D out AP
            # [d, sb, q] <- in [q, sb*128+d]); slot 0 = Q^T, 1 = K^T
            nc.sync.dma_start_transpose(out=qkt[:, :, slot, :], in_=src)

        # head 0: upfront, ordered so each xbar transpose fires as soon as
        # its tensor's cast lands; vn load AFTER the transposes (not needed
        # until the first AV, and its 1MB would delay them on the queue)
        kn0 = load_in(kap, 0, "kn")
        qn0 = load_in(qap, 0, "qn")
        kh0 = hpool.tile([128, SB, d], F16, tag="kh")
        qh0 = hpool.tile([128, SB, d], F16, tag="qh")
        qkt0 = tpool.tile([128, SB, 2, 128], F16, tag="qkt")
        nc.vector.tensor_copy(kh0, kn0)
        transpose_qkt(qkt0, 1, kh0)
        nc.vector.tensor_copy(qh0, qn0)
        transpose_qkt(qkt0, 0, qh0)
        vn0 = load_in(vap, 0, "vn")
        vp0 = vpool.tile([128, SB, d + 1], BF16, tag="vp")
        nc.vector.tensor_copy(vp0[:, :, 0:d], vn0)
        nc.vector.tensor_copy(vp0[:, :, d], invcb)
        cur = (qkt0, vp0)
        for bh in range(nbh):
            qkt, vp = cur

            expT = [epool.tile([128, s], BF16, tag=f"expT{kb}", name=f"expT{kb}_{bh}") for kb in range(SB)]
            ostage = opool.tile([128, SB, d], F32, tag="ostage")

            def av_block(qb):
                av = psav.tile([128, 256], F32, tag="av")
                for kb in range(qb + 1):
                    nc.tensor.matmul(
                        av[:, 0:d + 1],
                        expT[kb][:, qb * 128:(qb + 1) * 128],
                        vp[:, kb, :],
                        start=(kb == 0),
                        stop=(kb == qb),
                    )
                rec = small.tile([128, 1], F32, tag="rec")
                nc.vector.reciprocal(rec, av[:, d:d + 1])
                nc.vector.tensor_scalar_mul(ostage[:, qb, :], av[:, 0:d], rec)

            # scores strips capped at 1536 cols (3 PSUM banks) so two strip
            # slots + the av/transpose pool fit in the 8 PSUM banks; the long
            # strips (t < 4) are split into two slots/exps.
            for t in range(SB):
                for (lo, hi) in (((t * 128) // 512 * 512, min(((t * 128) // 512 * 512) + 1536, s)),
                                 (min(((t * 128) // 512 * 512) + 1536, s), s)):
                    if lo >= hi:
                        continue
                    sc = psum.tile([128, 1536], F32, tag="ps")
                    q0 = max(t * 128, lo)
                    qstart = q0
                    while qstart < hi:
                        seg = min(512 - (qstart % 512), hi - qstart)
                        b0, b1 = qstart // 128, (qstart + seg) // 128
                        diag = qstart == t * 128
                        nc.tensor.matmul(
                            sc[:, qstart - lo:qstart - lo + seg],
                            qkt[:, t, 1, :],
                            qkt[:, b0:b1, 0, :],
                            start=True,
                            stop=not diag,
                        )
                        if diag:
                            # accumulate -3e38 below the diagonal (PE-side
                            # causal mask, see trinegTb above)
                            nc.tensor.matmul(
                                sc[:, qstart - lo:qstart - lo + 128],
                                trinegTb,
                                identb,
                                start=False,
                                stop=True,
                                skip_group_check=True,
                            )
                        qstart += seg
                    # exp(s - 16 + ln(cm_key)) -> bf16
                    nc.scalar.activation(expT[t][:, q0:hi], sc[:, q0 - lo:hi - lo],
                                         EXPFN, bias=lncm[:, t:t + 1])
                if t >= 2:
                    # two steps behind: slack for exp AND the next head's
                    # V'/expT producers without idling the PE
                    av_block(t - 2)
                # Software pipeline for head bh+1, spread so no engine's
                # in-order queue parks long enough to starve the AV chain:
                # K/Q loads at t==2, V at t==4; fp16 casts in 1024-col DVE
                # chunks at t==8..13 (each delays at most one
                # reciprocal/scale pair); each xbar transpose right after
                # its tensor's last chunk.
                if bh + 1 < nbh:
                    if t == 2:
                        nxt_kn = load_in(kap, bh + 1, "kn")
                        nxt_qn = load_in(qap, bh + 1, "qn")
                        nxt_kh = hpool.tile([128, SB, d], F16, tag="kh")
                        nxt_qh = hpool.tile([128, SB, d], F16, tag="qh")
                        nxt_qkt = tpool.tile([128, SB, 2, 128], F16, tag="qkt")
                    elif t == 6:
                        cast_chunk(nxt_kh, nxt_kn, 0)
                    elif t == 7:
                        cast_chunk(nxt_kh, nxt_kn, 1)
                        # each xbar transpose barriers the whole DMA queue
                        # against in-flight DMAs: issue both transposes
                        # before the V load so they only barrier on the K/Q
                        # loads their casts need anyway
                        transpose_qkt(nxt_qkt, 1, nxt_kh)
                    elif t == 8:
                        cast_chunk(nxt_qh, nxt_qn, 0)
                    elif t == 9:
                        cast_chunk(nxt_qh, nxt_qn, 1)
                        transpose_qkt(nxt_qkt, 0, nxt_qh)
                    elif t == 10:
                        nxt_vn = load_in(vap, bh + 1, "vn")
                        nxt_vp = vpool.tile([128, SB, d + 1], BF16, tag="vp")
                    elif t == 13:
                        nc.vector.tensor_copy(nxt_vp[:, 0:SB // 2, 0:d],
                                              nxt_vn[:, 0:SB // 2, :])
                    elif t == 14:
                        nc.vector.tensor_copy(nxt_vp[:, SB // 2:, 0:d],
                                              nxt_vn[:, SB // 2:, :])
                        nc.vector.tensor_copy(nxt_vp[:, :, d], invcb)
                        cur = (nxt_qkt, nxt_vp)
            av_block(SB - 2)
            av_block(SB - 1)

            # chunked stores: all but the last chunk overlap compute; on the
            # sync queue AFTER the next head's loads/transposes already went
            # out (emitted mid-strip-loop above), so their last-AV-gated
            # waits can't delay the pipeline
            for g0 in range(0, SB, 4):
                gs = min(4, SB - g0)
                nc.sync.dma_start(
                    out=oap[bh][g0 * 128:(g0 + gs) * 128].rearrange(
                        "(sb p) d -> p sb d", p=128),
                    in_=ostage[:, g0:g0 + gs, :],
                )

    _legalize_waits(nc)
    return nc


_nc_cache = {}


def _get_nc():
    key = (NBH, S, D)
    if key not in _nc_cache:
        _nc_cache[key] = build_nc()
    return _nc_cache[key]


def kernel(query, key, value, ctx_mask):
    q = np.ascontiguousarray(query, dtype=np.float32).reshape(B * H, S, D)
    k = np.ascontiguousarray(key, dtype=np.float32).reshape(B * H, S, D)
    v = np.ascontiguousarray(value, dtype=np.float32).reshape(B * H, S, D)
    cmf = np.ascontiguousarray(ctx_mask, dtype=np.float32)

    in_maps = []
    for c in range(NCORES):
        lo = c * NBH
        in_maps.append({
            "q": q[lo:lo + NBH],
            "k": k[lo:lo + NBH],
            "v": v[lo:lo + NBH],
            "cm": cmf[(lo // H)],
        })
    nc = _get_nc()
    res = run_bass_kernel_spmd(nc, in_maps, list(range(NCORES)))
    outs = [res.results[c]["out"] for c in range(NCORES)]
    return np.concatenate(outs, axis=0).reshape(B, H, S, D).astype(np.float32)



# revision 20
# speedup vs baseline: 1.0687x; 1.0687x over previous
"""Trainium2 Bass kernel for GPT-Neo style causal attention.

reference:
    scores = q @ k.T              (no 1/sqrt(d) scaling), fp32
    scores = where(causal, scores, -inf)
    attn   = softmax(scores, -1)
    attn   = attn * ctx_mask[b, None, None, :]
    out    = attn @ v

Shapes: B=2, H=16, S=2048, D=128 fp32. Sharded over 8 cores by (b*h) —
4 heads per core; each core's heads belong to one batch, so one
ctx_mask row per core.

Per-core algorithm (T-layout softmax, no transposes of the attn matrix):
  - load Q,K natural fp32, GPSIMD-cast to fp16, then ONE xbar DMA
    transpose per tensor (3D-out batched 128x128 transpose) ->
    interleaved [Q^T | K^T] tile [d, s] in fp16 (1 cyc/col matmuls + FWL
    weight loads; frees the PE of 32 transpose matmuls/head vs a
    PE-transpose pipeline). The whole prep for head bh+1 is emitted
    BEFORE compute(bh): engines execute their queues in order, and the
    output stores (which wait on the last AV) must not block the next
    head's loads/transposes on the sync queue. Casts live on the
    otherwise-idle GPSIMD engine so the DVE (which feeds the per-strip
    reciprocal/scale chain) never parks on an input-DMA semaphore.
  - per key-block t: scoresT[keys,q] = KT_blk.T @ QT  (only q >= t*128,
    512-col segments aligned to PSUM banks)
  - one exp() per strip on ScalarE with per-partition bias ln(ctx_mask):
    expT = exp(s + ln(cm_key)) = exp(s)*cm_key  -> bf16 (the ctx-mask
    multiply costs nothing).  Causal diag via additive -3e38 mask on the
    diagonal block in PSUM pre-exp.
  - AV: out_psum[q, 0:129] = sum_kb expT_blk.T @ [V | 1/cm] (bf16,
    fp32 PSUM accum).  Column 128 accumulates exp*cm*(1/cm) = exp,
    i.e. the pre-ctx-mask softmax denominator -> reciprocal + scale.
  - cm clamped at 1e-30 so cm=0 stays exact (exp(s+ln(1e-30))*1e30 =
    exp(s) in the denominator, 0 in the numerator).

No max-subtraction is needed: |scores| <~ 95 so exp() stays inside fp32/
bf16 range after the -16 bias shift (which cancels in the softmax ratio).
A dummy bf16 matmul burst at the start (hidden under the first input
DMA + cast + transpose chain) warms the PE HAM clock gate to 2.4 GHz.
"""

from contextlib import ExitStack

import numpy as np

import concourse.bass as bass
import concourse.mybir as mybir
import concourse.tile as tile
from concourse.bass_utils import run_bass_kernel_spmd
from concourse.masks import make_identity, make_lower_triangular, make_upper_triangular

F32 = mybir.dt.float32
F32R = mybir.dt.float32r
F16 = mybir.dt.float16
BF16 = mybir.dt.bfloat16

B, H, S, D = 2, 16, 2048, 128
NCORES = 8
NBH = (B * H) // NCORES  # heads per core


def _legalize_waits(nc):
    """This container's walrus accepts at most 1 sync wait per instruction
    (2 for EventSemaphore). Hoist extra waits onto same-engine NoOps
    inserted immediately before the offending instruction (semantically
    identical: all waits still complete before it executes)."""
    n = 0
    ctr = [0]
    for f in nc.m.functions:
        for bb in f.blocks:
            out = []
            dirty = False
            for inst in bb.instructions:
                si = inst.sync_info
                cap = 2 if isinstance(inst, mybir.InstEventSemaphore) else 1
                if si is not None and len(si.on_wait) > cap:
                    waits = list(si.on_wait)
                    extra, keep = waits[:-cap], waits[-cap:]
                    for w in extra:
                        ctr[0] += 1
                        nop = mybir.InstNoOp(
                            name=f"waitsplit-{ctr[0]}",
                            ins=[],
                            outs=[],
                            engine=inst.engine,
                            sync_info=mybir.SyncInfo(on_wait=[w], on_update=[]),
                        )
                        nc.register_instruction(nop, overwrite=True)
                        out.append(nop)
                    inst.sync_info = mybir.SyncInfo(
                        on_wait=keep, on_update=list(si.on_update)
                    )
                    dirty = True
                    n += 1
                out.append(inst)
            if dirty:
                bb.instructions = out
    return n


def build_nc(nbh=NBH, s=S, d=D, num_devices=NCORES):
    SB = s // 128  # 128-row blocks along the sequence
    nc = bass.Bass("TRN2", target_bir_lowering=False, debug=False,
                   num_devices=num_devices)
    q = nc.dram_tensor("q", [nbh, s, d], F32, kind="ExternalInput")
    k = nc.dram_tensor("k", [nbh, s, d], F32, kind="ExternalInput")
    v = nc.dram_tensor("v", [nbh, s, d], F32, kind="ExternalInput")
    cm = nc.dram_tensor("cm", [s], F32, kind="ExternalInput")
    o = nc.dram_tensor("out", [nbh, s, d], F32, kind="ExternalOutput")

    EXPFN = mybir.ActivationFunctionType.Exp
    LNFN = mybir.ActivationFunctionType.Ln

    with tile.TileContext(nc) as tc, ExitStack() as ctx:
        consts = ctx.enter_context(tc.tile_pool(name="consts", bufs=1))
        stage = ctx.enter_context(tc.tile_pool(name="stage", bufs=2))
        hpool = ctx.enter_context(tc.tile_pool(name="hpool", bufs=2))
        tpool = ctx.enter_context(tc.tile_pool(name="tpool", bufs=2))
        vpool = ctx.enter_context(tc.tile_pool(name="vpool", bufs=2))
        epool = ctx.enter_context(tc.tile_pool(name="epool", bufs=1))
        opool = ctx.enter_context(tc.tile_pool(name="opool", bufs=2))
        small = ctx.enter_context(tc.tile_pool(name="small", bufs=4))
        psum = ctx.enter_context(tc.tile_pool(name="psum", bufs=2, space="PSUM"))
        psav = ctx.enter_context(tc.tile_pool(name="psav", bufs=2, space="PSUM"))

        ident = consts.tile([128, 128], F32)
        make_identity(nc, ident)
        tri32 = consts.tile([128, 128], F32)
        make_upper_triangular(nc, tri32, val=1.0, diag=True)
        tri = consts.tile([128, 128], BF16)
        nc.vector.tensor_copy(tri, tri32)
        # additive causal mask for the diagonal block, applied to the scores
        # BEFORE exp (a post-exp 0/1 multiply turns exp-overflow inf into NaN)
        trineg = consts.tile([128, 128], F32)
        make_lower_triangular(nc, trineg, val=-3e38, diag=False)

        # ctx-mask pipeline: cmc = max(cm, 1e-30); lncm = ln(cmc) (exp bias);
        # invc = 1/cmc in bf16 (denominator column of V')
        cmt = consts.tile([128, SB], F32)
        nc.sync.dma_start(out=cmt, in_=cm.ap().rearrange("(sb p) -> p sb", p=128))
        cmc = consts.tile([128, SB], F32)
        nc.vector.tensor_scalar_max(cmc, cmt, 1e-30)
        # -16 shift keeps exp() in fp32/bf16 range for the largest observed
        # scores (~95); it cancels exactly in the softmax ratio since the
        # denominator column scales identically.
        lncm = consts.tile([128, SB], F32)
        nc.scalar.activation(lncm, cmc, LNFN)
        nc.vector.tensor_scalar_add(lncm, lncm, -16.0)
        invc = consts.tile([128, SB], F32)
        nc.vector.reciprocal(invc, cmc)
        invcb = consts.tile([128, SB], BF16)
        nc.vector.tensor_copy(invcb, invc)

        # Dummy bf16 matmuls (values irrelevant) to warm the PE clock gate
        # while the first input DMAs are in flight; memset-only dep so the
        # burst starts at t~0.
        wpw = consts.tile([128, 128], BF16)
        nc.vector.memset(wpw, 1.0)
        wps = psav.tile([128, 256], F32, tag="av")
        for _ in range(130):
            nc.tensor.matmul(wps[:, 0:128], wpw, wpw, start=True, stop=True)

        qap, kap, vap, oap = q.ap(), k.ap(), v.ap(), o.ap()

        def prep(bh):
            """Emit loads + fp16 casts + xbar transposes + V' staging for
            head bh. Emitted one head AHEAD of compute(bh) so the sync
            queue isn't stuck behind the previous head's output stores
            (engines execute their program in order, and store DMAs block
            on the last AV's semaphore). K before Q: the first QK strip's
            weight is a K^T block. V last so its 1MB transfer doesn't
            delay the transposes on the queue (each xbar transpose
            barriers the DMA queue against all in-flight DMAs)."""
            kn = stage.tile([128, SB, d], F32, tag="kn")
            qn = stage.tile([128, SB, d], F32, tag="qn")
            nc.sync.dma_start(out=kn, in_=kap[bh].rearrange("(sb p) d -> p sb d", p=128))
            nc.sync.dma_start(out=qn, in_=qap[bh].rearrange("(sb p) d -> p sb d", p=128))
            kh = hpool.tile([128, SB, d], F16, tag="kh")
            qh = hpool.tile([128, SB, d], F16, tag="qh")
            nc.gpsimd.tensor_copy(kh, kn)
            nc.gpsimd.tensor_copy(qh, qn)
            # interleaved [Q^T | K^T] [d, s] fp16: one batched
            # 128x128-per-sb transpose DMA per tensor (3D out AP
            # [d, sb, q] <- in [q, sb*128+d]). qkt[:, sb, 0, :] = Q^T,
            # qkt[:, sb, 1, :] = K^T.
            qkt = tpool.tile([128, SB, 2, 128], F16, tag="qkt")
            nc.sync.dma_start_transpose(out=qkt[:, :, 1, :], in_=kh)
            nc.sync.dma_start_transpose(out=qkt[:, :, 0, :], in_=qh)

            # V' = [V | 1/cm] bf16; plain fp32 load, cast on GPSIMD
            vn = stage.tile([128, SB, d], F32, tag="vn")
            nc.sync.dma_start(out=vn,
                              in_=vap[bh].rearrange("(sb p) d -> p sb d", p=128))
            vp = vpool.tile([128, SB, d + 1], BF16, tag="vp")
            nc.gpsimd.tensor_copy(vp[:, :, 0:d], vn)
            nc.gpsimd.tensor_copy(vp[:, :, d], invcb)
            return qkt, vp

        nxt = prep(0)
        for bh in range(nbh):
            qkt, vp = nxt
            if bh + 1 < nbh:
                nxt = prep(bh + 1)

            expT = [epool.tile([128, s], BF16, tag=f"expT{kb}", name=f"expT{kb}_{bh}") for kb in range(SB)]
            ostage = opool.tile([128, SB, d], F32, tag="ostage")

            def av_block(qb):
                av = psav.tile([128, 256], F32, tag="av")
                for kb in range(qb + 1):
                    nc.tensor.matmul(
                        av[:, 0:d + 1],
                        expT[kb][:, qb * 128:(qb + 1) * 128],
                        vp[:, kb, :],
                        start=(kb == 0),
                        stop=(kb == qb),
                    )
                rec = small.tile([128, 1], F32, tag="rec")
                nc.vector.reciprocal(rec, av[:, d:d + 1])
                nc.vector.tensor_scalar_mul(ostage[:, qb, :], av[:, 0:d], rec)

            # scores strips capped at 1536 cols (3 PSUM banks) so two strip
            # slots + the av pool fit in the 8 PSUM banks; the long strips
            # (t < 4) are split into two slots/exps.
            for t in range(SB):
                for (lo, hi) in (((t * 128) // 512 * 512, min(((t * 128) // 512 * 512) + 1536, s)),
                                 (min(((t * 128) // 512 * 512) + 1536, s), s)):
                    if lo >= hi:
                        continue
                    sc = psum.tile([128, 1536], F32, tag="ps")
                    qstart = max(t * 128, lo)
                    while qstart < hi:
                        seg = min(512 - (qstart % 512), hi - qstart)
                        b0, b1 = qstart // 128, (qstart + seg) // 128
                        nc.tensor.matmul(
                            sc[:, qstart - lo:qstart - lo + seg],
                            qkt[:, t, 1, :],
                            qkt[:, b0:b1, 0, :],
                            start=True,
                            stop=True,
                        )
                        qstart += seg
                    q0 = max(t * 128, lo)
                    if q0 == t * 128:
                        # causal-mask the diagonal block in PSUM pre-exp
                        nc.vector.tensor_add(
                            sc[:, q0 - lo:q0 - lo + 128],
                            sc[:, q0 - lo:q0 - lo + 128],
                            trineg,
                        )
                    # exp(s - 16 + ln(cm_key)) -> bf16
                    nc.scalar.activation(expT[t][:, q0:hi], sc[:, q0 - lo:hi - lo],
                                         EXPFN, bias=lncm[:, t:t + 1])
                if t >= 1:
                    av_block(t - 1)  # one step behind so PE never waits on exp
            av_block(SB - 1)

            # chunked stores: all but the last chunk overlap compute
            for g0 in range(0, SB, 4):
                gs = min(4, SB - g0)
                nc.sync.dma_start(
                    out=oap[bh][g0 * 128:(g0 + gs) * 128].rearrange(
                        "(sb p) d -> p sb d", p=128),
                    in_=ostage[:, g0:g0 + gs, :],
                )

    _legalize_waits(nc)
    return nc


_nc_cache = {}


def _get_nc():
    key = (NBH, S, D)
    if key not in _nc_cache:
        _nc_cache[key] = build_nc()
    return _nc_cache[key]


def kernel(query, key, value, ctx_mask):
    q = np.ascontiguousarray(query, dtype=np.float32).reshape(B * H, S, D)
    k = np.ascontiguousarray(key, dtype=np.float32).reshape(B * H, S, D)
    v = np.ascontiguousarray(value, dtype=np.float32).reshape(B * H, S, D)
    cmf = np.ascontiguousarray(ctx_mask, dtype=np.float32)

    in_maps = []
    for c in range(NCORES):
        lo = c * NBH
        in_maps.append({
            "q": q[lo:lo + NBH],
            "k": k[lo:lo + NBH],
            "v": v[lo:lo + NBH],
            "cm": cmf[(lo // H)],
        })
    nc = _get_nc()
    res = run_bass_kernel_spmd(nc, in_maps, list(range(NCORES)))
    outs = [res.results[c]["out"] for c in range(NCORES)]
    return np.concatenate(outs, axis=0).reshape(B, H, S, D).astype(np.float32)


# revision 31
# speedup vs baseline: 1.1276x; 1.0551x over previous
"""Trainium2 Bass kernel for GPT-Neo style causal attention.

reference:
    scores = q @ k.T              (no 1/sqrt(d) scaling), fp32
    scores = where(causal, scores, -inf)
    attn   = softmax(scores, -1)
    attn   = attn * ctx_mask[b, None, None, :]
    out    = attn @ v

Shapes: B=2, H=16, S=2048, D=128 fp32. Sharded over 8 cores by (b*h) —
4 heads per core; each core's heads belong to one batch, so one
ctx_mask row per core.

Per-core algorithm (T-layout softmax, no transposes of the attn matrix):
  - load Q,K natural fp32, GPSIMD-cast to fp16, then ONE xbar DMA
    transpose per tensor (3D-out batched 128x128 transpose) ->
    interleaved [Q^T | K^T] tile [d, s] in fp16 (1 cyc/col matmuls + FWL
    weight loads; frees the PE of 32 transpose matmuls/head vs a
    PE-transpose pipeline). The whole prep for head bh+1 is emitted
    BEFORE compute(bh): engines execute their queues in order, and the
    output stores (which wait on the last AV) must not block the next
    head's loads/transposes on the sync queue. Casts live on the
    otherwise-idle GPSIMD engine so the DVE (which feeds the per-strip
    reciprocal/scale chain) never parks on an input-DMA semaphore.
  - per key-block t: scoresT[keys,q] = KT_blk.T @ QT  (only q >= t*128,
    512-col segments aligned to PSUM banks)
  - one exp() per strip on ScalarE with per-partition bias ln(ctx_mask):
    expT = exp(s + ln(cm_key)) = exp(s)*cm_key  -> bf16 (the ctx-mask
    multiply costs nothing).  Causal diag via additive -3e38 mask on the
    diagonal block in PSUM pre-exp.
  - AV: out_psum[q, 0:129] = sum_kb expT_blk.T @ [V | 1/cm] (bf16,
    fp32 PSUM accum).  Column 128 accumulates exp*cm*(1/cm) = exp,
    i.e. the pre-ctx-mask softmax denominator -> reciprocal + scale.
  - cm clamped at 1e-30 so cm=0 stays exact (exp(s+ln(1e-30))*1e30 =
    exp(s) in the denominator, 0 in the numerator).

No max-subtraction is needed: |scores| <~ 95 so exp() stays inside fp32/
bf16 range after the -16 bias shift (which cancels in the softmax ratio).
A dummy bf16 matmul burst at the start (hidden under the first input
DMA + cast + transpose chain) warms the PE HAM clock gate to 2.4 GHz.
"""

from contextlib import ExitStack

import numpy as np

import concourse.bass as bass
import concourse.mybir as mybir
import concourse.tile as tile
from concourse.bass_utils import run_bass_kernel_spmd
from concourse.masks import make_identity, make_lower_triangular, make_upper_triangular

F32 = mybir.dt.float32
F32R = mybir.dt.float32r
F16 = mybir.dt.float16
BF16 = mybir.dt.bfloat16

B, H, S, D = 2, 16, 2048, 128
NCORES = 8
NBH = (B * H) // NCORES  # heads per core


def _legalize_waits(nc):
    """This container's walrus accepts at most 1 sync wait per instruction
    (2 for EventSemaphore). Hoist extra waits onto same-engine NoOps
    inserted immediately before the offending instruction (semantically
    identical: all waits still complete before it executes)."""
    n = 0
    ctr = [0]
    for f in nc.m.functions:
        for bb in f.blocks:
            out = []
            dirty = False
            for inst in bb.instructions:
                si = inst.sync_info
                cap = 2 if isinstance(inst, mybir.InstEventSemaphore) else 1
                if si is not None and len(si.on_wait) > cap:
                    waits = list(si.on_wait)
                    extra, keep = waits[:-cap], waits[-cap:]
                    for w in extra:
                        ctr[0] += 1
                        nop = mybir.InstNoOp(
                            name=f"waitsplit-{ctr[0]}",
                            ins=[],
                            outs=[],
                            engine=inst.engine,
                            sync_info=mybir.SyncInfo(on_wait=[w], on_update=[]),
                        )
                        nc.register_instruction(nop, overwrite=True)
                        out.append(nop)
                    inst.sync_info = mybir.SyncInfo(
                        on_wait=keep, on_update=list(si.on_update)
                    )
                    dirty = True
                    n += 1
                out.append(inst)
            if dirty:
                bb.instructions = out
    return n


def build_nc(nbh=NBH, s=S, d=D, num_devices=NCORES):
    SB = s // 128  # 128-row blocks along the sequence
    nc = bass.Bass("TRN2", target_bir_lowering=False, debug=False,
                   num_devices=num_devices)
    q = nc.dram_tensor("q", [nbh, s, d], F32, kind="ExternalInput")
    k = nc.dram_tensor("k", [nbh, s, d], F32, kind="ExternalInput")
    v = nc.dram_tensor("v", [nbh, s, d], F32, kind="ExternalInput")
    cm = nc.dram_tensor("cm", [s], F32, kind="ExternalInput")
    o = nc.dram_tensor("out", [nbh, s, d], F32, kind="ExternalOutput")

    EXPFN = mybir.ActivationFunctionType.Exp
    LNFN = mybir.ActivationFunctionType.Ln

    with tile.TileContext(nc) as tc, ExitStack() as ctx:
        consts = ctx.enter_context(tc.tile_pool(name="consts", bufs=1))
        stage = ctx.enter_context(tc.tile_pool(name="stage", bufs=2))
        hpool = ctx.enter_context(tc.tile_pool(name="hpool", bufs=2))
        tpool = ctx.enter_context(tc.tile_pool(name="tpool", bufs=2))
        vpool = ctx.enter_context(tc.tile_pool(name="vpool", bufs=2))
        epool = ctx.enter_context(tc.tile_pool(name="epool", bufs=1))
        # 3 output buffers: head h's stores are emitted DURING head h+1
        # (after its loads, so their scale(15)-gated waits never park the
        # sync queue ahead of the loads); h+1's ostage writes must not WAR
        # on h-1's still-pending stores.
        opool = ctx.enter_context(tc.tile_pool(name="opool", bufs=3))
        small = ctx.enter_context(tc.tile_pool(name="small", bufs=4))
        psum = ctx.enter_context(tc.tile_pool(name="psum", bufs=2, space="PSUM"))
        psav = ctx.enter_context(tc.tile_pool(name="psav", bufs=2, space="PSUM"))

        ident = consts.tile([128, 128], F32)
        make_identity(nc, ident)
        identb = consts.tile([128, 128], BF16)
        nc.vector.tensor_copy(identb, ident)
        # additive causal mask for the diagonal block, accumulated into the
        # scores PSUM by the PE itself: matmul(trinegT, I) adds
        # trinegT.T[k, q] = -3e38 for q < k. Keeps the DVE off the
        # scores->exp critical path (a pre-exp DVE tensor_add there adds
        # DVE-queue latency to every strip's exp).
        trinegT = consts.tile([128, 128], F32)
        make_upper_triangular(nc, trinegT, val=-3e38, diag=False)
        trinegTb = consts.tile([128, 128], BF16)
        nc.vector.tensor_copy(trinegTb, trinegT)

        # ctx-mask pipeline: cmc = max(cm, 1e-30); lncm = ln(cmc) (exp bias);
        # invc = 1/cmc in bf16 (denominator column of V')
        cmt = consts.tile([128, SB], F32)
        nc.sync.dma_start(out=cmt, in_=cm.ap().rearrange("(sb p) -> p sb", p=128))
        cmc = consts.tile([128, SB], F32)
        nc.vector.tensor_scalar_max(cmc, cmt, 1e-30)
        # -16 shift keeps exp() in fp32/bf16 range for the largest observed
        # scores (~95); it cancels exactly in the softmax ratio since the
        # denominator column scales identically.
        lncm = consts.tile([128, SB], F32)
        nc.scalar.activation(lncm, cmc, LNFN)
        nc.vector.tensor_scalar_add(lncm, lncm, -16.0)
        invc = consts.tile([128, SB], F32)
        nc.vector.reciprocal(invc, cmc)
        invcb = consts.tile([128, SB], BF16)
        nc.vector.tensor_copy(invcb, invc)

        # Dummy bf16 matmuls (values irrelevant) to warm the PE clock gate
        # while the first input DMAs are in flight; memset-only dep so the
        # burst starts at t~0.
        wpw = consts.tile([128, 128], BF16)
        nc.vector.memset(wpw, 1.0)
        wps = psav.tile([128, 256], F32, tag="av")
        for _ in range(270):
            nc.tensor.matmul(wps[:, 0:128], wpw, wpw, start=True, stop=True)

        qap, kap, vap, oap = q.ap(), k.ap(), v.ap(), o.ap()

        def prep(bh, first=False):
            """Emit loads + fp16 casts + xbar transposes + V' staging for
            head bh. Emitted one head AHEAD of compute(bh) so the sync
            queue isn't stuck behind the previous head's output stores
            (engines execute their program in order, and store DMAs block
            on the last AV's semaphore). K before Q: the first QK strip's
            weight is a K^T block. V before the transposes: each xbar
            transpose barriers the DMA queue against all in-flight DMAs,
            and a V load issued after them would complete too late for
            the next head's first AV. K/Q casts live on the
            otherwise-idle GPSIMD (for head 0, on the then-idle DVE,
            which is ~4x faster per element); the V' cast is DVE chunks
            emitted late in the previous head's strip loop."""
            ceng = nc.vector if first else nc.gpsimd
            kn = stage.tile([128, SB, d], F32, tag="kn")
            qn = stage.tile([128, SB, d], F32, tag="qn")
            vn = stage.tile([128, SB, d], F32, tag="vn")
            nc.sync.dma_start(out=kn, in_=kap[bh].rearrange("(sb p) d -> p sb d", p=128))
            nc.sync.dma_start(out=qn, in_=qap[bh].rearrange("(sb p) d -> p sb d", p=128))
            nc.sync.dma_start(out=vn, in_=vap[bh].rearrange("(sb p) d -> p sb d", p=128))
            kh = hpool.tile([128, SB, d], F16, tag="kh")
            qh = hpool.tile([128, SB, d], F16, tag="qh")
            ceng.tensor_copy(kh, kn)
            ceng.tensor_copy(qh, qn)
            # interleaved [Q^T | K^T] [d, s] fp16: one batched
            # 128x128-per-sb transpose DMA per tensor (3D out AP
            # [d, sb, q] <- in [q, sb*128+d]). qkt[:, sb, 0, :] = Q^T,
            # qkt[:, sb, 1, :] = K^T.
            qkt = tpool.tile([128, SB, 2, 128], F16, tag="qkt")
            nc.sync.dma_start_transpose(out=qkt[:, :, 1, :], in_=kh)
            nc.sync.dma_start_transpose(out=qkt[:, :, 0, :], in_=qh)

            # V' = [V | 1/cm] bf16
            vp = vpool.tile([128, SB, d + 1], BF16, tag="vp")
            if first:
                nc.vector.tensor_copy(vp[:, :, 0:d], vn)
                nc.vector.tensor_copy(vp[:, :, d], invcb)
            return qkt, vp, vn

        def store_chunk(sbh, sostage, g0):
            nc.sync.dma_start(
                out=oap[sbh][g0 * 128:(g0 + SB // 2) * 128].rearrange(
                    "(sb p) d -> p sb d", p=128),
                in_=sostage[:, g0:g0 + SB // 2, :],
            )

        nxt = prep(0, first=True)
        prev = None
        for bh in range(nbh):
            qkt, vp, _ = nxt
            if bh + 1 < nbh:
                nxt = prep(bh + 1)

            expT = [epool.tile([128, s], BF16, tag=f"expT{kb}", name=f"expT{kb}_{bh}") for kb in range(SB)]
            ostage = opool.tile([128, SB, d], F32, tag="ostage")

            def av_block(qb):
                av = psav.tile([128, 256], F32, tag="av")
                for kb in range(qb + 1):
                    nc.tensor.matmul(
                        av[:, 0:d + 1],
                        expT[kb][:, qb * 128:(qb + 1) * 128],
                        vp[:, kb, :],
                        start=(kb == 0),
                        stop=(kb == qb),
                    )
                rec = small.tile([128, 1], F32, tag="rec")
                nc.vector.reciprocal(rec, av[:, d:d + 1])
                nc.vector.tensor_scalar_mul(ostage[:, qb, :], av[:, 0:d], rec)

            # scores strips capped at 1536 cols (3 PSUM banks) so two strip
            # slots + the av pool fit in the 8 PSUM banks; the long strips
            # (t < 4) are split into two slots/exps.
            for t in range(SB):
                for (lo, hi) in (((t * 128) // 512 * 512, min(((t * 128) // 512 * 512) + 1536, s)),
                                 (min(((t * 128) // 512 * 512) + 1536, s), s)):
                    if lo >= hi:
                        continue
                    sc = psum.tile([128, 1536], F32, tag="ps")
                    q0 = max(t * 128, lo)
                    qstart = q0
                    while qstart < hi:
                        seg = min(512 - (qstart % 512), hi - qstart)
                        b0, b1 = qstart // 128, (qstart + seg) // 128
                        diag = qstart == t * 128
                        nc.tensor.matmul(
                            sc[:, qstart - lo:qstart - lo + seg],
                            qkt[:, t, 1, :],
                            qkt[:, b0:b1, 0, :],
                            start=True,
                            stop=not diag,
                        )
                        if diag:
                            # accumulate -3e38 below the diagonal (PE-side
                            # causal mask, see trinegTb above)
                            nc.tensor.matmul(
                                sc[:, qstart - lo:qstart - lo + 128],
                                trinegTb,
                                identb,
                                start=False,
                                stop=True,
                                skip_group_check=True,
                            )
                        qstart += seg
                    # exp(s - 16 + ln(cm_key)) -> bf16
                    nc.scalar.activation(expT[t][:, q0:hi], sc[:, q0 - lo:hi - lo],
                                         EXPFN, bias=lncm[:, t:t + 1])
                if t >= 1:
                    av_block(t - 1)  # one step behind so PE never waits on exp
                # PREVIOUS head's stores, emitted here (after this head's
                # prep already went out on the sync queue) so their
                # scale-gated waits can never delay the loads/transposes;
                # both chunks' semaphores are long satisfied by now.
                if prev is not None:
                    if t == 1:
                        store_chunk(prev[0], prev[1], 0)
                    elif t == 3:
                        store_chunk(prev[0], prev[1], SB // 2)
                # next head's V' cast: late DVE chunks (its V load is done
                # by now, so the DVE never parks; reciprocal/scale slack
                # via the psav double-buffer absorbs the ~1.3us each)
                if bh + 1 < nbh:
                    if t == 13:
                        nc.vector.tensor_copy(nxt[1][:, 0:SB // 2, 0:d],
                                              nxt[2][:, 0:SB // 2, :])
                    elif t == 14:
                        nc.vector.tensor_copy(nxt[1][:, SB // 2:, 0:d],
                                              nxt[2][:, SB // 2:, :])
                        nc.vector.tensor_copy(nxt[1][:, :, d], invcb)
            av_block(SB - 1)
            prev = (bh, ostage)

        # last head's stores drain at the very end
        store_chunk(prev[0], prev[1], 0)
        store_chunk(prev[0], prev[1], SB // 2)

    _legalize_waits(nc)
    return nc


_nc_cache = {}


def _get_nc():
    key = (NBH, S, D)
    if key not in _nc_cache:
        _nc_cache[key] = build_nc()
    return _nc_cache[key]


def kernel(query, key, value, ctx_mask):
    q = np.ascontiguousarray(query, dtype=np.float32).reshape(B * H, S, D)
    k = np.ascontiguousarray(key, dtype=np.float32).reshape(B * H, S, D)
    v = np.ascontiguousarray(value, dtype=np.float32).reshape(B * H, S, D)
    cmf = np.ascontiguousarray(ctx_mask, dtype=np.float32)

    in_maps = []
    for c in range(NCORES):
        lo = c * NBH
        in_maps.append({
            "q": q[lo:lo + NBH],
            "k": k[lo:lo + NBH],
            "v": v[lo:lo + NBH],
            "cm": cmf[(lo // H)],
        })
    nc = _get_nc()
    res = run_bass_kernel_spmd(nc, in_maps, list(range(NCORES)))
    outs = [res.results[c]["out"] for c in range(NCORES)]
    return np.concatenate(outs, axis=0).reshape(B, H, S, D).astype(np.float32)
